# revision 12
# baseline (speedup 1.0000x reference)
"""Distributed multi-head attention (B=2, S=2048, D=1024, H=16) on 8 trn2 NeuronCores.

Sharding: data-parallel over batch (2) x tensor-parallel over heads (4 groups of 4
heads). Each core computes qkv for its 4 heads, full-sequence attention for those
heads, and a partial projection (its 256 columns of proj_w). The 4 partial proj
outputs per batch are summed on the host (unshard step); proj bias added on host.

Per-core kernel (all fp32, matmuls in fp32r):
  inputs : xT [D,S] (x[b] transposed), wqkT [D,512], wvT [D,256], projT [256,D]
  output : outT [D,S] = (attn_out_g @ proj_w[:,cols_g].T).T   (partial, pre-bias)

On-chip layout keeps q,k transposed ([dim, S]) so attention scores are computed
directly in the "scoresT" orientation [S_k, S_q]; softmax denominator comes from
an appended ones-column in the AV matmul's stationary operand; normalization is
a reciprocal + PE-broadcast + elementwise multiply at PSUM-eviction time.
"""

import os
import sys
import types

import numpy as np

# ----- problem constants (hardcoded; kernel.py must be self-contained) -----
B, S, D = 2, 2048, 1024
NH, HD = 16, 64
NCORES = 8
TP = 4                  # tensor-parallel degree (head groups) per batch
HL = NH // TP           # local heads per core = 4
LQK = 2 * HL * HD       # local q+k dims = 512
LV = HL * HD            # local v dims = 256
P = 128
QC = 512                # S_q chunk (one PSUM bank of fp32)

_NC = None
LAST_RESULTS = None


def _ensure_ntff_hook():
    """This image lacks antenv.axon_hooks; register the NTFF profile hook so
    run_bass_kernel_spmd(trace=True) can report HW exec time."""
    if "antenv.axon_hooks" in sys.modules:
        return
    try:
        from trn_agent_boot.trn_boot import _ntff_profile_via_ctypes

        hook = _ntff_profile_via_ctypes("/opt/axon/libaxon_pjrt.so")
    except Exception:
        hook = None
    mod = types.ModuleType("antenv.axon_hooks")
    mod.get_axon_ntff_profile_hook = lambda: hook
    sys.modules["antenv.axon_hooks"] = mod


def build_nc(S=S, Dm=D, QC=QC, debug_dumps=False):
    """Emit the per-core Bass program (identical on all 8 cores)."""
    import concourse.bacc as bacc
    import concourse.tile as tile
    from concourse import mybir

    f32 = mybir.dt.float32
    f32r = mybir.dt.float32r
    Exp = mybir.ActivationFunctionType.Exp
    SCALE = HD ** -0.5

    KT = S // P          # S_k partition tiles
    DK = Dm // P         # contraction tiles over model dim
    NQ = S // QC         # S_q chunks
    MV = LV // P         # v / attn-out partition tiles (2)
    MQ = LQK // P        # q+k dim tiles (4): 0..MV-1 = q, MV..MQ-1 = k
    HW1 = HD + 1         # v columns + ones column

    nc = bacc.Bacc("TRN2", target_bir_lowering=False, debug=False)
    xT = nc.dram_tensor("xT", [Dm, S], f32r, kind="ExternalInput").ap()
    wqkT = nc.dram_tensor("wqkT", [Dm, LQK], f32r, kind="ExternalInput").ap()
    wvT = nc.dram_tensor("wvT", [Dm, LV], f32r, kind="ExternalInput").ap()
    projT = nc.dram_tensor("projT", [LV, Dm], f32r, kind="ExternalInput").ap()
    ones_d = nc.dram_tensor("ones", [P, HL], f32r, kind="ExternalInput").ap()
    if debug_dumps:
        qk_dbg = nc.dram_tensor("qk_dbg", [MQ * P, S], f32, kind="ExternalOutput").ap()
        vug_dbg = nc.dram_tensor("vug_dbg", [KT * P, HL * HW1], f32, kind="ExternalOutput").ap()
        at_dbg = nc.dram_tensor("at_dbg", [MV * P, S], f32, kind="ExternalOutput").ap()
        rec_dbg = nc.dram_tensor("rec_dbg", [HL, QC], f32, kind="ExternalOutput").ap()
        pav_dbg = nc.dram_tensor("pav_dbg", [HL * HW1, QC], f32, kind="ExternalOutput").ap()
    outT = nc.dram_tensor("outT", [Dm, S], f32, kind="ExternalOutput").ap()

    def r(ap):
        return ap.bitcast(f32r)

    with tile.TileContext(nc) as tc:
        from contextlib import ExitStack

        with ExitStack() as ctx:
            xp_pool = ctx.enter_context(tc.tile_pool(name="xp", bufs=DK))
            wqk_pool = ctx.enter_context(tc.tile_pool(name="wqk", bufs=DK))
            wv_pool = ctx.enter_context(tc.tile_pool(name="wv", bufs=DK))
            qk_pool = ctx.enter_context(tc.tile_pool(name="qk", bufs=MQ))
            vug_pool = ctx.enter_context(tc.tile_pool(name="vug", bufs=KT))
            at_pool = ctx.enter_context(tc.tile_pool(name="at", bufs=MV))
            pj_pool = ctx.enter_context(tc.tile_pool(name="pj", bufs=MV))
            on_pool = ctx.enter_context(tc.tile_pool(name="on", bufs=1))
            ex_pool = ctx.enter_context(tc.tile_pool(name="ex", bufs=5))
            rc_pool = ctx.enter_context(tc.tile_pool(name="rc", bufs=2))
            rb_pool = ctx.enter_context(tc.tile_pool(name="rb", bufs=2))
            st_pool = ctx.enter_context(tc.tile_pool(name="st", bufs=2))
            pm_pool = ctx.enter_context(tc.tile_pool(name="pm", bufs=2, space="PSUM"))
            sc_pool = ctx.enter_context(tc.tile_pool(name="sc", bufs=2, space="PSUM"))
            av_pool = ctx.enter_context(tc.tile_pool(name="av", bufs=2, space="PSUM"))

            # ---- loads ----
            xp = []
            wqk = []
            wv = []
            for k in range(DK):
                wt = wqk_pool.tile([P, LQK], f32r)
                nc.sync.dma_start(wt[:], wqkT[k * P : (k + 1) * P, :])
                wqk.append(wt)
                xt = xp_pool.tile([P, S], f32r)
                nc.sync.dma_start(xt[:], xT[k * P : (k + 1) * P, :])
                xp.append(xt)
            for k in range(DK):
                vt = wv_pool.tile([P, LV], f32r)
                nc.sync.dma_start(vt[:], wvT[k * P : (k + 1) * P, :])
                wv.append(vt)
            pj = []
            for j in range(MV):
                pt = pj_pool.tile([P, Dm], f32r)
                nc.sync.dma_start(pt[:], projT[j * P : (j + 1) * P, :])
                pj.append(pt)
            on1 = on_pool.tile([1, HD], f32)
            nc.gpsimd.memset(on1[:], 1.0)
            vug = []
            for kt in range(KT):
                vt = vug_pool.tile([P, HL * HW1], f32r)
                ones_cols = vt.rearrange("p (h w) -> p h w", w=HW1)[:, :, HD]
                nc.sync.dma_start(ones_cols, ones_d[:])
                vug.append(vt)

            qk = [
                qk_pool.tile([P, S], f32r, name="qk", tag="qk") for _ in range(MQ)
            ]
            at = [
                at_pool.tile([P, S], f32r, name="at", tag="at") for _ in range(MV)
            ]

            # ---- k-dim tiles of qkT (m = MV..MQ-1) ----
            for m in range(MV, MQ):
                for n in range(NQ):
                    ps = pm_pool.tile([P, QC], f32, name="pm", tag="pm")
                    for k in range(DK):
                        nc.tensor.matmul(
                            ps[:],
                            lhsT=wqk[k][:, m * P : (m + 1) * P],
                            rhs=xp[k][:, n * QC : (n + 1) * QC],
                            start=(k == 0),
                            stop=(k == DK - 1),
                        )
                    nc.vector.tensor_copy(qk[m][:, n * QC : (n + 1) * QC], ps[:])

            # ---- v tiles (natural [S, LV] layout) into vug (with ones col) ----
            for mt in range(KT):
                ps = pm_pool.tile([P, LV], f32, name="pm", tag="pm")
                for k in range(DK):
                    nc.tensor.matmul(
                        ps[:],
                        lhsT=xp[k][:, mt * P : (mt + 1) * P],
                        rhs=wv[k][:],
                        start=(k == 0),
                        stop=(k == DK - 1),
                    )
                for h in range(HL):
                    nc.vector.tensor_copy(
                        vug[mt][:, h * HW1 : h * HW1 + HD],
                        ps[:, h * HD : (h + 1) * HD],
                    )

            # ---- q-dim tiles of qkT (m = 0..MV-1) ----
            for n in range(NQ):
                for m in range(MV):
                    ps = pm_pool.tile([P, QC], f32, name="pm", tag="pm")
                    for k in range(DK):
                        nc.tensor.matmul(
                            ps[:],
                            lhsT=wqk[k][:, m * P : (m + 1) * P],
                            rhs=xp[k][:, n * QC : (n + 1) * QC],
                            start=(k == 0),
                            stop=(k == DK - 1),
                        )
                    nc.vector.tensor_copy(qk[m][:, n * QC : (n + 1) * QC], ps[:])

            if debug_dumps:
                for m in range(MQ):
                    nc.sync.dma_start(qk_dbg[m * P : (m + 1) * P, :], qk[m].bitcast(f32))
                for kt in range(KT):
                    nc.sync.dma_start(vug_dbg[kt * P : (kt + 1) * P, :], vug[kt].bitcast(f32))

            # ---- attention + partial projection, per S_q chunk ----
            for qc in range(NQ):
                for h in range(HL):
                    hh = h % 2
                    qt = qk[h // 2]
                    ktile = qk[MV + h // 2]
                    pav = av_pool.tile([HW1, QC], f32)
                    for g in range(KT // 2):
                        scp = sc_pool.tile([P, 2 * QC], f32)
                        for j in range(2):
                            kt = 2 * g + j
                            nc.tensor.matmul(
                                scp[:, j * QC : (j + 1) * QC],
                                lhsT=ktile[hh * HD : (hh + 1) * HD, kt * P : (kt + 1) * P],
                                rhs=qt[hh * HD : (hh + 1) * HD, qc * QC : (qc + 1) * QC],
                                start=True,
                                stop=True,
                            )
                        ext = ex_pool.tile([P, 2 * QC], f32r)
                        nc.scalar.activation(ext[:], scp[:], Exp, scale=SCALE)
                        for j in range(2):
                            kt = 2 * g + j
                            nc.tensor.matmul(
                                pav[:],
                                lhsT=vug[kt][:, h * HW1 : (h + 1) * HW1],
                                rhs=ext[:, j * QC : (j + 1) * QC],
                                start=(kt == 0),
                                stop=(kt == KT - 1),
                            )
                    den = rc_pool.tile([1, QC], f32, name="den", tag="den")
                    nc.vector.tensor_copy(den[:], pav[HD : HD + 1, :])
                    rec = rc_pool.tile([1, QC], f32)
                    nc.vector.reciprocal_approx_fast(rec[:], den[:])
                    pb = pm_pool.tile([HD, QC], f32, name="pm", tag="pm")
                    nc.tensor.matmul(
                        pb[:], lhsT=on1[:], rhs=rec[:], start=True, stop=True
                    )
                    if debug_dumps and qc == 0:
                        pv = st_pool.tile([HW1, QC], f32, name="pvd", tag="pvd")
                        nc.vector.tensor_copy(pv[:], pav[:])
                        nc.sync.dma_start(pav_dbg[h * HW1 : (h + 1) * HW1, :], pv[:])
                        nc.sync.dma_start(rec_dbg[h : h + 1, :], rec[:])
                    rb = rb_pool.tile([HD, QC], f32)
                    nc.vector.tensor_copy(rb[:], pb[:])
                    nc.vector.tensor_mul(
                        at[h // 2][hh * HD : (hh + 1) * HD, qc * QC : (qc + 1) * QC],
                        pav[0:HD, :],
                        rb[:],
                    )
                if debug_dumps and qc == NQ - 1:
                    for j in range(MV):
                        nc.sync.dma_start(at_dbg[j * P : (j + 1) * P, :], at[j].bitcast(f32))
                # partial projection for this chunk
                for m in range(DK):
                    pp = pm_pool.tile([P, QC], f32, name="pm", tag="pm")
                    for j in range(MV):
                        nc.tensor.matmul(
                            pp[:],
                            lhsT=pj[j][:, m * P : (m + 1) * P],
                            rhs=at[j][:, qc * QC : (qc + 1) * QC],
                            start=(j == 0),
                            stop=(j == MV - 1),
                        )
                    stg = st_pool.tile([P, QC], f32)
                    nc.vector.tensor_copy(stg[:], pp[:])
                    nc.sync.dma_start(
                        outT[m * P : (m + 1) * P, qc * QC : (qc + 1) * QC], stg[:]
                    )

    nc.compile()
    return nc


def _get_nc():
    global _NC
    if _NC is None:
        _NC = build_nc()
    return _NC


def _shard_inputs(x, qkv_w):
    """Per-core DRAM inputs, sharding per (batch, head-group)."""
    maps = []
    xTs = [np.ascontiguousarray(x[b].T) for b in range(B)]
    for c in range(NCORES):
        b, g = divmod(c, TP)
        rows_q = qkv_w[g * LV : (g + 1) * LV]
        rows_k = qkv_w[D + g * LV : D + (g + 1) * LV]
        wqkT = np.ascontiguousarray(np.concatenate([rows_q, rows_k], 0).T)
        wvT = np.ascontiguousarray(qkv_w[2 * D + g * LV : 2 * D + (g + 1) * LV].T)
        maps.append(
            {
                "xT": xTs[b],
                "wqkT": wqkT,
                "wvT": wvT,
                "ones": np.ones((P, HL), dtype=np.float32),
            }
        )
    return maps


def kernel(**inputs):
    x = np.asarray(inputs["x"], dtype=np.float32)
    qkv_w = np.asarray(inputs["qkv_w"], dtype=np.float32)
    proj_w = np.asarray(inputs["proj_w"], dtype=np.float32)
    proj_b = np.asarray(inputs["proj_b"], dtype=np.float32)
    # xpos is unused by the reference (rope disabled)

    _ensure_ntff_hook()
    nc = _get_nc()
    from concourse.bass_utils import run_bass_kernel_spmd

    in_maps = _shard_inputs(x, qkv_w)
    for c in range(NCORES):
        g = c % TP
        in_maps[c]["projT"] = np.ascontiguousarray(
            proj_w[:, g * LV : (g + 1) * LV].T
        )

    trace = os.environ.get("BASS_KERNEL_TRACE") == "1"
    res = run_bass_kernel_spmd(nc, in_maps, list(range(NCORES)), trace=trace)
    global LAST_RESULTS
    LAST_RESULTS = res

    out = np.zeros((B, S, D), dtype=np.float32)
    for c in range(NCORES):
        b = c // TP
        out[b] += res.results[c]["outT"].T
    out += proj_b[None, None, :]
    return out
